# revision 1
# baseline (speedup 1.0000x reference)
"""Bass/Trainium2 kernel for nn_Attention_75007308857927.

Reference computation (B=4, S=2048, D=1024):
    Q = X @ Wq.T ; K = X @ Wk.T ; V = X @ Wv.T         (per batch)
    Qn, Kn = row-normalized Q, K
    scores = (Qn @ Kn.T) * m      m[i,j] = 1 if (j > i) or masks[j]==0 else 0
    out = scores @ V

Sharding: 8 cores = 4 batches x 2 query/key-halves. Each core projects
K/V/Q from its own 1024-row half of X; the full KT and V' (V scaled by
1/||K||) are assembled across the core pair with an AllGather, then each
core computes scores/out for its query half.

Device algebra per core (matmuls contract over the partition dim):
    KT[e,j']  = sum_d WkT[d,e] * XQ[d,j']        (own keys j', KT e-major)
    kinv[j']  = rsqrt(sum_e KT[e,j']^2)
    V'[j',e]  = (sum_d XQ[d,j'] WvT[d,e]) * kinv[j']
    KT, V'    = AllGather over the core pair     (global key order)
    QT[e,i]   = sum_d WqT[d,e] * XQ[d,i]
    qinv[i]   = rsqrt(sum_e QT[e,i]^2)
    ST[j,i]   = (sum_e KT[e,j] QT[e,i]) * maskT[j,i]   (mask fused in evict)
    out[i,d]  = (sum_j ST[j,i] V'[j,d]) * qinv[i]

bf16 matmul operands, f32 PSUM accumulation. Precision vs f32 reference:
absmax error ~0.4% of output scale.
"""

import numpy as np
import ml_dtypes

B, S, D = 4, 2048, 1024
HALF = S // 2  # queries/keys per core
N_CORES = 8
P = 128
DC = D // P    # 8 contraction chunks over d
ET = D // P    # 8 e-tiles
JT = S // P    # 16 j-tiles (global)
JTH = HALF // P  # 8 own j-tiles
I5 = HALF // 512  # 2

BF16 = ml_dtypes.bfloat16

_CACHE = {}


def _emit(ctx, tc, xq, wkt, wvt, wqt, maskt, out, kt_own, kt_gath, v_own, v_gath):
    from concourse import mybir

    nc = tc.nc
    dtb = mybir.dt.bfloat16
    dtf = mybir.dt.float32

    # ---- SBUF pools -------------------------------------------------------
    xq_p = ctx.enter_context(tc.tile_pool(name="xq", bufs=1))
    # weights + scores blocks share one pool: all tiles are 16KB/partition
    w_p = ctx.enter_context(tc.tile_pool(name="wst", bufs=3))
    kt_p = ctx.enter_context(tc.tile_pool(name="kt", bufs=1))
    qt_p = ctx.enter_context(tc.tile_pool(name="qt", bufs=1))
    vp_p = ctx.enter_context(tc.tile_pool(name="vp", bufs=1))
    row_p = ctx.enter_context(tc.tile_pool(name="rows", bufs=1))
    sq_p = ctx.enter_context(tc.tile_pool(name="sq", bufs=3))
    stg_p = ctx.enter_context(tc.tile_pool(name="stg", bufs=6))
    mk_p = ctx.enter_context(tc.tile_pool(name="mk", bufs=8))
    ev_p = ctx.enter_context(tc.tile_pool(name="ev", bufs=3))
    ps_p = ctx.enter_context(tc.tile_pool(name="psmm", bufs=5, space="PSUM"))
    psr_p = ctx.enter_context(tc.tile_pool(name="psrow", bufs=2, space="PSUM"))
    psc_p = ctx.enter_context(tc.tile_pool(name="pscol", bufs=1, space="PSUM"))

    xq_s = xq_p.tile([P, DC * HALF], dtb, tag="xq")    # [d%128, dc*1024+i]
    wkt_s = w_p.tile([P, DC * D], dtb, tag="w")        # [d%128, dc*1024+e]
    wvt_s = w_p.tile([P, DC * D], dtb, tag="w")
    wqt_s = w_p.tile([P, DC * D], dtb, tag="w")
    kt_s = kt_p.tile([P, ET * S], dtb, tag="kt")       # [e%128, et*2048+j]
    qt_s = qt_p.tile([P, ET * HALF], dtb, tag="qt")    # [e%128, et*1024+i]
    vp_s = vp_p.tile([P, JT * D], dtb, tag="vp")       # [j%128, jt*1024+d]

    ones_b = row_p.tile([P, 1], dtb, tag="ones_b")
    ones_f = row_p.tile([1, 1], dtf, tag="ones_f")
    ksq_row = row_p.tile([1, HALF], dtf, tag="sqrow")
    qsq_row = row_p.tile([1, HALF], dtf, tag="sqrow")
    ksq_col = row_p.tile([P, JTH], dtf, tag="ksqc")    # col c <-> own j-chunk c
    krec_col = row_p.tile([P, JTH], dtf, tag="krecc")
    kinv_col = row_p.tile([P, JTH], dtf, tag="kinvc")
    qsq_col = row_p.tile([P, ET], dtf, tag="qsqc")
    qrec_col = row_p.tile([P, ET], dtf, tag="qrecc")
    qinv_col = row_p.tile([P, ET], dtf, tag="qinvc")

    nc.vector.memset(ones_b[:], 1.0)
    nc.vector.memset(ones_f[:], 1.0)

    # ---- input DMAs (ordered for earliest matmul start: B needs wkt+xq) --
    for dc in range(DC):
        nc.sync.dma_start(wkt_s[:, dc * D:(dc + 1) * D], wkt[dc * P:(dc + 1) * P, :])
        nc.scalar.dma_start(xq_s[:, dc * HALF:(dc + 1) * HALF],
                            xq[dc * P:(dc + 1) * P, :])
    for w_s, w_d in ((wvt_s, wvt), (wqt_s, wqt)):
        for dc in range(DC):
            nc.sync.dma_start(w_s[:, dc * D:(dc + 1) * D], w_d[dc * P:(dc + 1) * P, :])

    groups = [[0, 1], [2, 3], [4, 5], [6, 7]]

    # ---- phase B: KT for own keys + k sumsq ------------------------------
    for j5 in range(I5):
        ksq_ps = psr_p.tile([1, 512], dtf, tag="psrow")
        for et in range(ET):
            ps = ps_p.tile([P, 512], dtf, tag="psmm")
            for dc in range(DC):
                nc.tensor.matmul(
                    ps[:],
                    lhsT=wkt_s[:, dc * D + et * P: dc * D + (et + 1) * P],
                    rhs=xq_s[:, dc * HALF + j5 * 512: dc * HALF + j5 * 512 + 512],
                    start=(dc == 0), stop=(dc == DC - 1),
                )
            stg = stg_p.tile([P, 512], dtb, tag="stg")
            nc.vector.tensor_copy(stg[:], ps[:])
            nc.sync.dma_start(kt_own[et * P:(et + 1) * P, j5 * 512: j5 * 512 + 512],
                              stg[:])
            sq = sq_p.tile([P, 512], dtb, tag="sq")
            nc.scalar.square(sq[:], ps[:])
            nc.tensor.matmul(ksq_ps[:], lhsT=ones_b[:], rhs=sq[:],
                             start=(et == 0), stop=(et == ET - 1))
        nc.vector.tensor_copy(ksq_row[0:1, j5 * 512: j5 * 512 + 512], ksq_ps[:])
        for cc in range(4):
            c = j5 * 4 + cc
            pc = psc_p.tile([P, 1], dtf, tag="pscol")
            nc.tensor.matmul(pc[:], lhsT=ksq_row[0:1, c * P:(c + 1) * P],
                             rhs=ones_f[:], start=True, stop=True)
            nc.vector.tensor_copy(ksq_col[:, c:c + 1], pc[:])

    # kinv for own keys, 128-way parallel in column layout
    nc.vector.reciprocal(krec_col[:], ksq_col[:])
    nc.scalar.sqrt(kinv_col[:], krec_col[:])

    # gather KT across the core pair, then load full KT to SBUF
    nc.gpsimd.collective_compute(
        "AllGather", mybir.AluOpType.bypass, replica_groups=groups,
        ins=[kt_own[:]], outs=[kt_gath[:]])
    kt3 = kt_s[:].rearrange("p (et j) -> p et j", et=ET, j=S)
    for r in range(2):
        src3 = kt_gath[r].rearrange("(et p) j -> p et j", p=P)
        for eg in range(0, ET, 2):
            nc.gpsimd.dma_start(
                kt3[:, eg:eg + 2, r * HALF:(r + 1) * HALF],
                src3[:, eg:eg + 2, :])

    # ---- phase D: V' = V * kinv[j] for own keys --------------------------
    # e5 pair shares the stationary xq tile per dc step (weight reuse)
    for jt in range(JTH):
        ps_a = ps_p.tile([P, 512], dtf, tag="psmm")
        ps_b = ps_p.tile([P, 512], dtf, tag="psmm")
        pspair = [ps_a, ps_b]
        for dc in range(DC):
            for e5 in range(2):
                nc.tensor.matmul(
                    pspair[e5][:],
                    lhsT=xq_s[:, dc * HALF + jt * P: dc * HALF + (jt + 1) * P],
                    rhs=wvt_s[:, dc * D + e5 * 512: dc * D + e5 * 512 + 512],
                    start=(dc == 0), stop=(dc == DC - 1),
                )
        for e5 in range(2):
            stg = stg_p.tile([P, 512], dtb, tag="stg")
            nc.vector.tensor_scalar_mul(stg[:], pspair[e5][:], kinv_col[:, jt:jt + 1])
            nc.sync.dma_start(v_own[jt * P:(jt + 1) * P, e5 * 512: e5 * 512 + 512],
                              stg[:])

    nc.gpsimd.collective_compute(
        "AllGather", mybir.AluOpType.bypass, replica_groups=groups,
        ins=[v_own[:]], outs=[v_gath[:]])
    for r in range(2):
        dst = vp_s[:, r * JTH * D: (r * JTH + JTH) * D]
        dst = dst.rearrange("p (jtl e) -> p jtl e", jtl=JTH, e=D)
        src_ap = v_gath[r].rearrange("(jtl p) e -> p jtl e", p=P)
        for jg in range(0, JTH, 2):
            nc.gpsimd.dma_start(dst[:, jg:jg + 2, :], src_ap[:, jg:jg + 2, :])

    # ---- phase E: QT + q sumsq -------------------------------------------
    for i5 in range(I5):
        qsq_ps = psr_p.tile([1, 512], dtf, tag="psrow")
        for et in range(ET):
            ps = ps_p.tile([P, 512], dtf, tag="psmm")
            for dc in range(DC):
                nc.tensor.matmul(
                    ps[:],
                    lhsT=wqt_s[:, dc * D + et * P: dc * D + (et + 1) * P],
                    rhs=xq_s[:, dc * HALF + i5 * 512: dc * HALF + i5 * 512 + 512],
                    start=(dc == 0), stop=(dc == DC - 1),
                )
            qtsl = qt_s[:, et * HALF + i5 * 512: et * HALF + i5 * 512 + 512]
            nc.vector.tensor_copy(qtsl, ps[:])
            sq = sq_p.tile([P, 512], dtb, tag="sq")
            nc.scalar.square(sq[:], ps[:])
            nc.tensor.matmul(qsq_ps[:], lhsT=ones_b[:], rhs=sq[:],
                             start=(et == 0), stop=(et == ET - 1))
        nc.vector.tensor_copy(qsq_row[0:1, i5 * 512: i5 * 512 + 512], qsq_ps[:])

    # ---- phase F: all score blocks first (hides the V' gather), then ------
    # ---- phase G: all out blocks ------------------------------------------
    st_blks = []
    for ib in range(I5):
        st_blk = w_p.tile([P, JT * 512], dtb, tag="w")  # [j%128, jt*512+i]
        st_blks.append(st_blk)
        for jt in range(JT):
            ps = ps_p.tile([P, 512], dtf, tag="psmm")
            for et in range(ET):
                nc.tensor.matmul(
                    ps[:],
                    lhsT=kt_s[:, et * S + jt * P: et * S + (jt + 1) * P],
                    rhs=qt_s[:, et * HALF + ib * 512: et * HALF + ib * 512 + 512],
                    start=(et == 0), stop=(et == ET - 1),
                )
            mk = mk_p.tile([P, 512], dtb, tag="mk")
            nc.sync.dma_start(mk[:], maskt[jt * P:(jt + 1) * P,
                                           ib * 512: ib * 512 + 512])
            nc.vector.tensor_mul(st_blk[:, jt * 512:(jt + 1) * 512], ps[:], mk[:])
    # q-norm chain (deferred so F's matmuls aren't blocked behind it)
    for c in range(ET):
        pc = psc_p.tile([P, 1], dtf, tag="pscol")
        nc.tensor.matmul(pc[:], lhsT=qsq_row[0:1, c * P:(c + 1) * P],
                         rhs=ones_f[:], start=True, stop=True)
        nc.vector.tensor_copy(qsq_col[:, c:c + 1], pc[:])
    nc.vector.reciprocal(qrec_col[:], qsq_col[:])
    nc.scalar.sqrt(qinv_col[:], qrec_col[:])

    for ib in range(I5):
        st_blk = st_blks[ib]
        for itl in range(4):
            g = ib * 4 + itl  # global i-tile
            for d5 in range(2):
                ps = ps_p.tile([P, 512], dtf, tag="psmm")
                for jt in range(JT):
                    nc.tensor.matmul(
                        ps[:],
                        lhsT=st_blk[:, jt * 512 + itl * P: jt * 512 + (itl + 1) * P],
                        rhs=vp_s[:, jt * D + d5 * 512: jt * D + d5 * 512 + 512],
                        start=(jt == 0), stop=(jt == JT - 1),
                    )
                ot = ev_p.tile([P, 512], dtf, tag="ev")
                nc.vector.tensor_scalar_mul(ot[:], ps[:], qinv_col[:, g:g + 1])
                nc.sync.dma_start(out[g * P:(g + 1) * P, d5 * 512: d5 * 512 + 512],
                                  ot[:])


def _build():
    if "nc" in _CACHE:
        return _CACHE["nc"]
    import concourse.tile as tile
    from concourse import bacc, mybir

    dtb = mybir.dt.bfloat16
    dtf = mybir.dt.float32
    nc = bacc.Bacc("TRN2", target_bir_lowering=False, debug=False,
                   enable_asserts=True, num_devices=N_CORES)
    xq = nc.dram_tensor("xq", [D, HALF], dtb, kind="ExternalInput").ap()
    wkt = nc.dram_tensor("wkt", [D, D], dtb, kind="ExternalInput").ap()
    wvt = nc.dram_tensor("wvt", [D, D], dtb, kind="ExternalInput").ap()
    wqt = nc.dram_tensor("wqt", [D, D], dtb, kind="ExternalInput").ap()
    maskt = nc.dram_tensor("maskt", [S, HALF], dtb, kind="ExternalInput").ap()
    out = nc.dram_tensor("out", [HALF, D], dtf, kind="ExternalOutput").ap()
    kt_own = nc.dram_tensor("kt_own", [D, HALF], dtb).ap()
    kt_gath = nc.dram_tensor("kt_gath", [2, D, HALF], dtb).ap()
    v_own = nc.dram_tensor("v_own", [HALF, D], dtb).ap()
    v_gath = nc.dram_tensor("v_gath", [2, HALF, D], dtb).ap()

    from contextlib import ExitStack
    with tile.TileContext(nc) as tc:
        with ExitStack() as ctx:
            _emit(ctx, tc, xq, wkt, wvt, wqt, maskt, out,
                  kt_own, kt_gath, v_own, v_gath)
    nc.compile()
    _CACHE["nc"] = nc
    return nc


def make_in_maps(X, masks, Wq, Wk, Wv):
    """Host-side sharding/layout: one input map per core (global key order)."""
    in_maps = []
    wkt_h = np.ascontiguousarray(Wk.T).astype(BF16)
    wvt_h = np.ascontiguousarray(Wv.T).astype(BF16)
    wqt_h = np.ascontiguousarray(Wq.T).astype(BF16)
    for c in range(N_CORES):
        b, h = c // 2, c % 2
        XT = X[b].T.astype(BF16)                                # [D, S]
        j = np.arange(S)[:, None]
        i = h * HALF + np.arange(HALF)[None, :]
        mT = ((j > i) | (masks[b] == 0)[:, None]).astype(BF16)  # [S, HALF]
        in_maps.append({
            "xq": np.ascontiguousarray(XT[:, h * HALF:(h + 1) * HALF]),
            "wkt": wkt_h,
            "wvt": wvt_h,
            "wqt": wqt_h,
            "maskt": mT,
        })
    return in_maps


def run(in_maps, **kw):
    from concourse.bass_utils import run_bass_kernel_spmd
    nc = _build()
    return run_bass_kernel_spmd(nc, in_maps, list(range(N_CORES)), **kw)


def kernel(X, masks, Wq, Wk, Wv):
    X = np.asarray(X, dtype=np.float32)
    masks = np.asarray(masks)
    res = run(make_in_maps(X, masks, np.asarray(Wq, np.float32),
                           np.asarray(Wk, np.float32), np.asarray(Wv, np.float32)))
    out = np.empty((B, S, D), np.float32)
    for c in range(N_CORES):
        b, h = c // 2, c % 2
        out[b, h * HALF:(h + 1) * HALF, :] = res.results[c]["out"]
    return out



# revision 2
# speedup vs baseline: 1.0468x; 1.0468x over previous
"""Bass/Trainium2 kernel for nn_Attention_75007308857927 (v2).

Reference computation (B=4, S=2048, D=1024):
    Q = X @ Wq.T ; K = X @ Wk.T ; V = X @ Wv.T         (per batch)
    Qn, Kn = row-normalized Q, K
    scores = (Qn @ Kn.T) * m      m[i,j] = 1 if (j > i) or masks[j]==0 else 0
    out = scores @ V

NOTE the reference keeps scores on FUTURE positions (j > i) and PADDED
keys (masks[j]==0) — the mask is the complement of standard attention.
So padded keys (~half) contribute densely to every query, while valid
keys contribute anti-causally (only to queries i < j).

v2 strategy on top of the v1 pair-sharded design:
  * Host-side key reordering per batch: [padded keys asc ++ valid keys
    asc]. Key tiles below the padded/valid boundary are mask-free
    (dense); valid-key tiles are skipped for query tiles entirely in
    their past (anti-causal block skipping). Loop bounds are derived
    from the actual masks at runtime (max over cores, SPMD-uniform);
    per-core differences live in DMA'd mask data.
  * Query tiles are interleaved across the core pair (h=0:
    [0,2,4,6,9,11,13,15]) to balance the anti-causal work.
  * Sumsq reductions (for 1/||K||, 1/||Q||) are deferred behind each
    projection phase so the matmul stream has no ACT-dependency stalls.
  * Both AllGather doorbells are issued on gpsimd before any dependent
    SBUF loads; gather loads go on separate DMA queues so the second
    collective starts as soon as the CC stream frees up.

Device algebra per core (matmuls contract over the partition dim):
    KT[e,p']  = sum_d WkT[d,e] * XK[d,p']        (own reordered keys p')
    kinv[p']  = rsqrt(sum_e KT[e,p']^2)
    V'[p',e]  = (sum_d XK[d,p'] WvT[d,e]) * kinv[p']
    KT, V'    = AllGather over the core pair     (global reordered keys)
    QT[e,i]   = sum_d WqT[d,e] * XQ[d,i]         (own queries, slot order)
    qinv[i]   = rsqrt(sum_e QT[e,i]^2)
    ST[p,i]   = (sum_e KT[e,p] QT[e,i]) * mask[p,i]
    out[i,d]  = (sum_p ST[p,i] V'[p,d]) * qinv[i]

bf16 matmul operands, f32 PSUM accumulation.
"""

import numpy as np
import ml_dtypes

B, S, D = 4, 2048, 1024
HALF = S // 2          # queries / keys per core
N_CORES = 8
P = 128
DC = D // P            # 8 contraction chunks over d
ET = D // P            # 8 e-tiles
JT = S // P            # 16 global key tiles
JTH = HALF // P        # 8 own key tiles

BF16 = ml_dtypes.bfloat16

# query-tile assignment: balanced anti-causal load, sorted ascending
OWN_TILES = [
    [0, 2, 4, 6, 9, 11, 13, 15],   # h = 0
    [1, 3, 5, 7, 8, 10, 12, 14],   # h = 1
]

_CACHE = {}


def _emit(ctx, tc, cfg, xq, xk, wkt, wvt, wqt, maskt, out, cc):
    from concourse import mybir

    JA, TCOPY, GSTART = cfg
    FSTART = (min(GSTART[0:4]), min(GSTART[4:8]))
    f_tiles = [list(range(JA)) + list(range(FSTART[s], JT)) for s in range(2)]
    st_pos = [{t: i for i, t in enumerate(f_tiles[s])} for s in range(2)]
    NT = [len(f_tiles[s]) for s in range(2)]
    mrow = {}
    r = 0
    for s in range(2):
        for t in f_tiles[s]:
            if t >= TCOPY:
                mrow[(s, t)] = r
                r += 1
    NM = r

    nc = tc.nc
    dtb = mybir.dt.bfloat16
    dtf = mybir.dt.float32

    # ---- SBUF pools -------------------------------------------------------
    big_p = ctx.enter_context(tc.tile_pool(name="big", bufs=4))
    xq_p = ctx.enter_context(tc.tile_pool(name="xq", bufs=1))
    qt_p = ctx.enter_context(tc.tile_pool(name="qt", bufs=1))
    kt_p = ctx.enter_context(tc.tile_pool(name="kt", bufs=1))
    vp_p = ctx.enter_context(tc.tile_pool(name="vp", bufs=1))
    row_p = ctx.enter_context(tc.tile_pool(name="rows", bufs=1))
    sq_p = ctx.enter_context(tc.tile_pool(name="sq", bufs=9))
    stg_p = ctx.enter_context(tc.tile_pool(name="stg", bufs=6))
    mk_p = ctx.enter_context(tc.tile_pool(name="mk", bufs=7))
    ev_p = ctx.enter_context(tc.tile_pool(name="ev", bufs=4))
    # 7 main matmul banks + 1 bank for the row/col reductions
    ps_p = ctx.enter_context(tc.tile_pool(name="psmm", bufs=7, space="PSUM"))
    psr_p = ctx.enter_context(tc.tile_pool(name="psrow", bufs=1, space="PSUM"))

    wkt_s = big_p.tile([P, DC * D], dtb, tag="big")    # [d%128, dc*1024+e]
    wvt_s = big_p.tile([P, DC * D], dtb, tag="big")
    wqt_s = big_p.tile([P, DC * D], dtb, tag="big")
    xk_s = big_p.tile([P, DC * HALF], dtb, tag="big")  # [d%128, dc*1024+p']
    xq_s = xq_p.tile([P, DC * HALF], dtb, tag="xq")    # [d%128, dc*1024+i]
    qt_s = qt_p.tile([P, ET * HALF], dtb, tag="qt")    # [e%128, et*1024+i]
    kt_s = kt_p.tile([P, ET * S], dtb, tag="kt")       # [e%128, et*2048+p]
    vp_s = vp_p.tile([P, JT * D], dtb, tag="vp")       # [p%128, pt*1024+d]

    ones_b = row_p.tile([P, 1], dtb, tag="ones_b")
    ones_f = row_p.tile([1, 1], dtb, tag="ones_f")
    ksq_row = row_p.tile([1, HALF], dtb, tag="sqrow")
    qsq_row = row_p.tile([1, HALF], dtb, tag="sqrow2")
    ksq_col = row_p.tile([P, JTH], dtf, tag="ksqc")
    krec_col = row_p.tile([P, JTH], dtf, tag="krecc")
    kinv_col = row_p.tile([P, JTH], dtf, tag="kinvc")
    qsq_col = row_p.tile([P, ET], dtf, tag="qsqc")
    qrec_col = row_p.tile([P, ET], dtf, tag="qrecc")
    qinv_col = row_p.tile([P, ET], dtf, tag="qinvc")

    nc.vector.memset(ones_b[:], 1.0)
    nc.vector.memset(ones_f[:], 1.0)

    # ---- input DMAs -------------------------------------------------------
    # sync: wkt dc0-3, then kt stores / wqt / masks / kt-gather loads /
    #   even out stores. scalar: wkt dc4-7, wvt, xq, v stores, vp-a loads,
    #   odd out stores. gpsimd: xk (j5=0 halves first), doorbells,
    #   kt-gather + vp-b loads.
    wkt_s3 = wkt_s[:].rearrange("p (dc e) -> p dc e", dc=DC)
    wkt3d = wkt.rearrange("(dc p) e -> p dc e", p=P)
    nc.sync.dma_start(wkt_s3[:, 0, 0:256], wkt3d[:, 0, 0:256])
    nc.sync.dma_start(wkt_s3[:, 0, 256:1024], wkt3d[:, 0, 256:1024])
    for dc in range(1, 4):
        nc.sync.dma_start(wkt_s3[:, dc, :], wkt3d[:, dc, :])
    for dc in range(4, DC):
        nc.scalar.dma_start(wkt_s3[:, dc, :], wkt3d[:, dc, :])
    xk_s3 = xk_s[:].rearrange("p (dc h j) -> p dc h j", dc=DC, h=2)
    xk4d = xk.rearrange("(dc p) (h j) -> p dc h j", p=P, h=2)
    for dc in range(DC):
        nc.gpsimd.dma_start(xk_s3[:, dc, 0, :], xk4d[:, dc, 0, :])
    for dc in range(DC):
        nc.sync.dma_start(xk_s3[:, dc, 1, :], xk4d[:, dc, 1, :])
    wvt3d = wvt.rearrange("(dc p) e -> p dc e", p=P)
    wvt_s3 = wvt_s[:].rearrange("p (dc e) -> p dc e", dc=DC)
    nc.gpsimd.dma_start(wvt_s3[:, 0:4, :], wvt3d[:, 0:4, :])
    nc.scalar.dma_start(wvt_s3[:, 4:8, :], wvt3d[:, 4:8, :])
    xq3d = xq.rearrange("(dc p) e -> p dc e", p=P)
    xq_s3 = xq_s[:].rearrange("p (dc e) -> p dc e", dc=DC)
    for dc in range(DC):
        nc.scalar.dma_start(xq_s3[:, dc, :], xq3d[:, dc, :])

    groups = [[0, 1], [2, 3], [4, 5], [6, 7]]

    # ---- phase B: KT for own keys + k sumsq -------------------------------
    # dc-outer wave over et0-6 (7 banks) so matmuls start with the first
    # input chunk; et7 runs dc-inner right after; j5=0's reductions
    # interleave into j5=1's groups.
    sq_k = []

    def b_evict(j5, et, ps):
        stg = stg_p.tile([P, 512], dtb, tag="stg", name=f"stgb{j5}_{et}")
        nc.vector.tensor_copy(stg[:], ps[:])
        nc.sync.dma_start(cc[f"kt_own{j5}"][et * P:(et + 1) * P, :], stg[:])
        sq = sq_p.tile([P, 512], dtb, tag="sq", name=f"sqb{j5}_{et}")
        nc.scalar.square(sq[:], ps[:])
        sq_k.append(sq)

    def b_group(j5, et, interleave=None):
        ps = ps_p.tile([P, 512], dtf, tag="psmm", name=f"bg{j5}_{et}")
        for dc in range(DC):
            nc.tensor.matmul(
                ps[:],
                lhsT=wkt_s[:, dc * D + et * P: dc * D + (et + 1) * P],
                rhs=xk_s[:, dc * HALF + j5 * 512: dc * HALF + j5 * 512 + 512],
                start=(dc == 0), stop=(dc == DC - 1),
            )
        if interleave is not None:
            interleave()
        b_evict(j5, et, ps)

    ps_wave = [ps_p.tile([P, 512], dtf, tag="psmm", name=f"bw{et}")
               for et in range(7)]
    for dc in range(DC):
        for et in range(7):
            nc.tensor.matmul(
                ps_wave[et][:],
                lhsT=wkt_s[:, dc * D + et * P: dc * D + (et + 1) * P],
                rhs=xk_s[:, dc * HALF: dc * HALF + 512],
                start=(dc == 0), stop=(dc == DC - 1),
            )
    for et in range(7):
        b_evict(0, et, ps_wave[et])
    b_group(0, 7)

    # first-half gather doorbell fires as soon as j5=0's stores land
    nc.gpsimd.collective_compute(
        "AllGather", mybir.AluOpType.bypass, replica_groups=groups,
        ins=[cc["kt_own0"][:]], outs=[cc["kt_gath0"][:]])

    ksq_ps0 = psr_p.tile([1, 512], dtf, tag="psrow")
    ps_wave1 = [ps_p.tile([P, 512], dtf, tag="psmm", name=f"bx{et}")
                for et in range(7)]
    for dc in range(DC):
        for et in range(7):
            nc.tensor.matmul(
                ps_wave1[et][:],
                lhsT=wkt_s[:, dc * D + et * P: dc * D + (et + 1) * P],
                rhs=xk_s[:, dc * HALF + 512: dc * HALF + 1024],
                start=(dc == 0), stop=(dc == DC - 1),
            )
        if dc < 7:
            nc.tensor.matmul(ksq_ps0[:], lhsT=ones_b[:], rhs=sq_k[dc][:],
                             start=(dc == 0), stop=False)
    for et in range(7):
        b_evict(1, et, ps_wave1[et])

    def il7():
        nc.tensor.matmul(ksq_ps0[:], lhsT=ones_b[:], rhs=sq_k[7][:],
                         start=False, stop=True)
    b_group(1, 7, il7)
    nc.vector.tensor_copy(ksq_row[0:1, 0:512], ksq_ps0[:])
    ksq_ps1 = psr_p.tile([1, 512], dtf, tag="psrow")
    for et in range(ET):
        nc.tensor.matmul(ksq_ps1[:], lhsT=ones_b[:], rhs=sq_k[ET + et][:],
                         start=(et == 0), stop=(et == ET - 1))
    nc.vector.tensor_copy(ksq_row[0:1, 512:1024], ksq_ps1[:])
    for c in range(JTH):
        pc = psr_p.tile([P, 1], dtf, tag="psrow", name=f"kpc{c}")
        nc.tensor.matmul(pc[:], lhsT=ksq_row[0:1, c * P:(c + 1) * P],
                         rhs=ones_f[:], start=True, stop=True)
        nc.vector.tensor_copy(ksq_col[:, c:c + 1], pc[:])
    nc.vector.reciprocal(krec_col[:], ksq_col[:])
    nc.scalar.sqrt(kinv_col[:], krec_col[:])

    nc.gpsimd.collective_compute(
        "AllGather", mybir.AluOpType.bypass, replica_groups=groups,
        ins=[cc["kt_own1"][:]], outs=[cc["kt_gath1"][:]])

    # wqt on scalar (behind xq)
    wqt3d = wqt.rearrange("(dc p) e -> p dc e", p=P)
    wqt_s3 = wqt_s[:].rearrange("p (dc e) -> p dc e", dc=DC)
    for dc in range(DC):
        nc.scalar.dma_start(wqt_s3[:, dc, :], wqt3d[:, dc, :])

    # masks on sync behind the kt stores
    mk_tiles = []
    m3 = maskt.rearrange("(r p) i -> p r i", p=P)
    for r0 in range(0, NM, 2):
        r1 = min(r0 + 2, NM)
        mk = mk_p.tile([P, 1024], dtb, tag="mk", name=f"mk{r0}")
        mk3 = mk[:].rearrange("p (two i) -> p two i", two=2)
        nc.sync.dma_start(mk3[:, 0:r1 - r0, :], m3[:, r0:r1, :])
        mk_tiles.append(mk)

    # ---- phase D: V' = V * kinv for own keys ------------------------------
    for jt in range(JTH):
        ps_a = ps_p.tile([P, 512], dtf, tag="psmm")
        ps_b = ps_p.tile([P, 512], dtf, tag="psmm")
        pspair = [ps_a, ps_b]
        for dc in range(DC):
            for e5 in range(2):
                nc.tensor.matmul(
                    pspair[e5][:],
                    lhsT=xk_s[:, dc * HALF + jt * P: dc * HALF + (jt + 1) * P],
                    rhs=wvt_s[:, dc * D + e5 * 512: dc * D + e5 * 512 + 512],
                    start=(dc == 0), stop=(dc == DC - 1),
                )
        half = jt // 4
        for e5 in range(2):
            stg = stg_p.tile([P, 512], dtb, tag="stg", name=f"stgv{jt}_{e5}")
            nc.vector.tensor_scalar_mul(stg[:], pspair[e5][:], kinv_col[:, jt:jt + 1])
            nc.scalar.dma_start(
                cc[f"v_own{half}"][(jt % 4) * P:(jt % 4 + 1) * P,
                                   e5 * 512: e5 * 512 + 512], stg[:])
        if jt == 3:
            nc.gpsimd.collective_compute(
                "AllGather", mybir.AluOpType.bypass, replica_groups=groups,
                ins=[cc["v_own0"][:]], outs=[cc["v_gath0"][:]])
    nc.gpsimd.collective_compute(
        "AllGather", mybir.AluOpType.bypass, replica_groups=groups,
        ins=[cc["v_own1"][:]], outs=[cc["v_gath1"][:]])

    # kt SBUF loads (1MB per (half, rank)): r0 on sync, r1 on gpsimd
    kt3 = kt_s[:].rearrange("p (et j) -> p et j", et=ET, j=S)
    for h5 in range(2):
        gath = cc[f"kt_gath{h5}"]
        for r in range(2):
            src3 = gath[r].rearrange("(et p) j -> p et j", p=P)
            dst = kt3[:, :, r * HALF + h5 * 512: r * HALF + h5 * 512 + 512]
            eng = nc.sync if r == 0 else nc.gpsimd
            eng.dma_start(dst, src3)

    # ---- phase E: QT + q sumsq -------------------------------------------
    sq_q = []

    def e_evict(i5, et, ps):
        qtsl = qt_s[:, et * HALF + i5 * 512: et * HALF + i5 * 512 + 512]
        nc.vector.tensor_copy(qtsl, ps[:])
        sq = sq_p.tile([P, 512], dtb, tag="sq", name=f"sqe{i5}_{et}")
        nc.scalar.square(sq[:], ps[:])
        sq_q.append(sq)

    def e_group(i5, et, interleave=None):
        ps = ps_p.tile([P, 512], dtf, tag="psmm", name=f"eg{i5}_{et}")
        for dc in range(DC):
            nc.tensor.matmul(
                ps[:],
                lhsT=wqt_s[:, dc * D + et * P: dc * D + (et + 1) * P],
                rhs=xq_s[:, dc * HALF + i5 * 512: dc * HALF + i5 * 512 + 512],
                start=(dc == 0), stop=(dc == DC - 1),
            )
        if interleave is not None:
            interleave()
        e_evict(i5, et, ps)

    ps_wave2 = [ps_p.tile([P, 512], dtf, tag="psmm", name=f"ew{et}")
                for et in range(7)]
    for dc in range(DC):
        for et in range(7):
            nc.tensor.matmul(
                ps_wave2[et][:],
                lhsT=wqt_s[:, dc * D + et * P: dc * D + (et + 1) * P],
                rhs=xq_s[:, dc * HALF: dc * HALF + 512],
                start=(dc == 0), stop=(dc == DC - 1),
            )
    for et in range(7):
        e_evict(0, et, ps_wave2[et])
    e_group(0, 7)

    qsq_ps0 = psr_p.tile([1, 512], dtf, tag="psrow")
    for et in range(ET):
        def il(et=et):
            nc.tensor.matmul(qsq_ps0[:], lhsT=ones_b[:], rhs=sq_q[et][:],
                             start=(et == 0), stop=(et == ET - 1))
        e_group(1, et, il)
    nc.vector.tensor_copy(qsq_row[0:1, 0:512], qsq_ps0[:])
    qsq_ps1 = psr_p.tile([1, 512], dtf, tag="psrow")

    q_aux = [("ones", et) for et in range(ET)]
    q_aux.append(("rowcopy", None))
    q_aux += [("transpose", c) for c in range(ET)]

    def emit_aux():
        kind, a = q_aux.pop(0)
        if kind == "ones":
            nc.tensor.matmul(qsq_ps1[:], lhsT=ones_b[:], rhs=sq_q[ET + a][:],
                             start=(a == 0), stop=(a == ET - 1))
        elif kind == "rowcopy":
            nc.vector.tensor_copy(qsq_row[0:1, 512:1024], qsq_ps1[:])
        else:
            pc = psr_p.tile([P, 1], dtf, tag="psrow", name=f"qpc{a}")
            nc.tensor.matmul(pc[:], lhsT=qsq_row[0:1, a * P:(a + 1) * P],
                             rhs=ones_f[:], start=True, stop=True)
            nc.vector.tensor_copy(qsq_col[:, a:a + 1], pc[:])

    # vp SBUF loads: one 1MB DMA per (half, rank), split scalar/gpsimd
    vp3 = vp_s[:].rearrange("p (jtl e) -> p jtl e", jtl=JT, e=D)
    for h5 in range(2):
        gath = cc[f"v_gath{h5}"]
        for r in range(2):
            src_ap = gath[r].rearrange("(jtl p) e -> p jtl e", p=P)
            dst = vp3[:, r * JTH + h5 * 4: r * JTH + h5 * 4 + 4, :]
            eng = nc.scalar if h5 == 0 else nc.gpsimd
            eng.dma_start(dst, src_ap)

    # ---- phase F: score blocks (dense padded part + anti-causal part) -----
    st_blks = [big_p.tile([P, NT[s] * 512], dtb, tag="big", name=f"st_blk{s}")
               for s in range(2)]
    for s in range(2):
        st_blk = st_blks[s]
        for t in f_tiles[s]:
            pos = st_pos[s][t]
            ps = ps_p.tile([P, 512], dtf, tag="psmm")
            for et in range(ET):
                nc.tensor.matmul(
                    ps[:],
                    lhsT=kt_s[:, et * S + t * P: et * S + (t + 1) * P],
                    rhs=qt_s[:, et * HALF + s * 512: et * HALF + s * 512 + 512],
                    start=(et == 0), stop=(et == ET - 1),
                )
            if q_aux:
                emit_aux()
            dst = st_blk[:, pos * 512:(pos + 1) * 512]
            if t < TCOPY:
                nc.vector.tensor_copy(dst, ps[:])
            else:
                r = mrow[(s, t)]
                mk = mk_tiles[r // 2]
                nc.vector.tensor_mul(dst, ps[:], mk[:, (r % 2) * 512:
                                                    (r % 2) * 512 + 512])
    while q_aux:
        emit_aux()
    nc.vector.reciprocal(qrec_col[:], qsq_col[:])
    nc.scalar.sqrt(qinv_col[:], qrec_col[:])

    # ---- phase G: out = (ST^T @ V') * qinv --------------------------------
    for s in range(2):
        st_blk = st_blks[s]
        for c in range(4):
            k = 4 * s + c
            g_tiles = list(range(JA)) + list(range(GSTART[k], JT))
            for d5 in range(2):
                ps = ps_p.tile([P, 512], dtf, tag="psmm")
                for n, t in enumerate(g_tiles):
                    pos = st_pos[s][t]
                    nc.tensor.matmul(
                        ps[:],
                        lhsT=st_blk[:, pos * 512 + c * P: pos * 512 + (c + 1) * P],
                        rhs=vp_s[:, t * D + d5 * 512: t * D + d5 * 512 + 512],
                        start=(n == 0), stop=(n == len(g_tiles) - 1),
                    )
                ot = ev_p.tile([P, 512], dtf, tag="ev")
                nc.vector.tensor_scalar_mul(ot[:], ps[:], qinv_col[:, k:k + 1])
                eng = nc.sync if d5 == 0 else nc.scalar
                eng.dma_start(out[k * P:(k + 1) * P, d5 * 512: d5 * 512 + 512],
                              ot[:])


def _build(cfg):
    if cfg in _CACHE:
        return _CACHE[cfg]
    import concourse.tile as tile
    from concourse import bacc, mybir

    JA, TCOPY, GSTART = cfg
    FSTART = (min(GSTART[0:4]), min(GSTART[4:8]))
    f_tiles = [list(range(JA)) + list(range(FSTART[s], JT)) for s in range(2)]
    NM = sum(sum(1 for t in f_tiles[s] if t >= TCOPY) for s in range(2))

    dtb = mybir.dt.bfloat16
    dtf = mybir.dt.float32
    nc = bacc.Bacc("TRN2", target_bir_lowering=False, debug=False,
                   enable_asserts=True, num_devices=N_CORES)
    xq = nc.dram_tensor("xq", [D, HALF], dtb, kind="ExternalInput").ap()
    xk = nc.dram_tensor("xk", [D, HALF], dtb, kind="ExternalInput").ap()
    wkt = nc.dram_tensor("wkt", [D, D], dtb, kind="ExternalInput").ap()
    wvt = nc.dram_tensor("wvt", [D, D], dtb, kind="ExternalInput").ap()
    wqt = nc.dram_tensor("wqt", [D, D], dtb, kind="ExternalInput").ap()
    maskt = nc.dram_tensor("maskt", [max(NM, 1) * P, 512], dtb,
                           kind="ExternalInput").ap()
    out = nc.dram_tensor("out", [HALF, D], dtf, kind="ExternalOutput").ap()
    cc = {}
    for h5 in range(2):
        cc[f"kt_own{h5}"] = nc.dram_tensor(f"kt_own{h5}", [D, 512], dtb).ap()
        cc[f"kt_gath{h5}"] = nc.dram_tensor(f"kt_gath{h5}", [2, D, 512], dtb).ap()
        cc[f"v_own{h5}"] = nc.dram_tensor(f"v_own{h5}", [512, D], dtb).ap()
        cc[f"v_gath{h5}"] = nc.dram_tensor(f"v_gath{h5}", [2, 512, D], dtb).ap()

    from contextlib import ExitStack
    with tile.TileContext(nc) as tc:
        with ExitStack() as ctx:
            _emit(ctx, tc, cfg, xq, xk, wkt, wvt, wqt, maskt, out, cc)
    nc.compile()
    _CACHE[cfg] = nc
    return nc


def plan(masks):
    """Derive key reorder + skip bounds from the masks (SPMD-uniform)."""
    masks = np.asarray(masks)
    packed, nAs = [], []
    for b in range(B):
        iA = np.flatnonzero(masks[b] == 0)   # padded: always visible
        iV = np.flatnonzero(masks[b] != 0)   # valid: visible iff j > i
        packed.append(np.concatenate([iA, iV]).astype(np.int64))
        nAs.append(len(iA))
    JA = max(max(-(-n // P) for n in nAs), 1)
    TCOPY = min(n // P for n in nAs)
    GSTART = []
    for k in range(8):
        st = JT
        for b in range(B):
            pk, nA = packed[b], nAs[b]
            for h in range(2):
                g = OWN_TILES[h][k]
                t = JA
                while t < JT:
                    p = np.arange(t * P, (t + 1) * P)
                    mv = np.where(p >= nA, pk[p], -1).max()
                    if mv > g * P:
                        break
                    t += 1
                st = min(st, t)
        GSTART.append(st)
    # slots ascend in tile index -> bounds must be non-increasing suffixes
    for k in range(6, -1, -1):
        GSTART[k] = min(GSTART[k], GSTART[k + 1])
    return (JA, TCOPY, tuple(GSTART)), packed, nAs


def make_in_maps(X, masks, Wq, Wk, Wv):
    """Host-side key reorder + layout: one input map per core."""
    cfg, packed, nAs = plan(masks)
    JA, TCOPY, GSTART = cfg
    FSTART = (min(GSTART[0:4]), min(GSTART[4:8]))
    f_tiles = [list(range(JA)) + list(range(FSTART[s], JT)) for s in range(2)]
    NM = sum(sum(1 for t in f_tiles[s] if t >= TCOPY) for s in range(2))
    in_maps = []
    wkt_h = np.ascontiguousarray(Wk.T).astype(BF16)
    wvt_h = np.ascontiguousarray(Wv.T).astype(BF16)
    wqt_h = np.ascontiguousarray(Wq.T).astype(BF16)
    for c in range(N_CORES):
        b, h = c // 2, c % 2
        XT = X[b].T.astype(BF16)                                # [D, S]
        pk, nA = packed[b], nAs[b]
        own_keys = pk[h * HALF:(h + 1) * HALF]
        tiles = OWN_TILES[h]
        qrows = np.concatenate([np.arange(g * P, (g + 1) * P) for g in tiles])
        # mask blocks, [NM*128, 512]: rows = global reordered key pos,
        # cols = own queries in slot order
        mt = np.zeros((max(NM, 1) * P, 4 * P), BF16)
        cols = [np.concatenate([np.arange(tiles[4 * s + cc] * P,
                                          (tiles[4 * s + cc] + 1) * P)
                                for cc in range(4)]) for s in range(2)]
        r = 0
        for s in range(2):
            for t in f_tiles[s]:
                if t < TCOPY:
                    continue
                p = t * P + np.arange(P)
                is_pad = p < nA
                vis = is_pad[:, None] | (pk[p][:, None] > cols[s][None, :])
                mt[r * P:(r + 1) * P, :] = vis
                r += 1
        in_maps.append({
            "xq": np.ascontiguousarray(XT[:, qrows]),
            "xk": np.ascontiguousarray(XT[:, own_keys]),
            "wkt": wkt_h,
            "wvt": wvt_h,
            "wqt": wqt_h,
            "maskt": mt,
        })
    return in_maps, cfg


def run(in_maps, cfg, **kw):
    from concourse.bass_utils import run_bass_kernel_spmd
    nc = _build(cfg)
    return run_bass_kernel_spmd(nc, in_maps, list(range(N_CORES)), **kw)


def kernel(X, masks, Wq, Wk, Wv):
    X = np.asarray(X, dtype=np.float32)
    masks = np.asarray(masks)
    in_maps, cfg = make_in_maps(X, masks, np.asarray(Wq, np.float32),
                                np.asarray(Wk, np.float32),
                                np.asarray(Wv, np.float32))
    res = run(in_maps, cfg)
    out = np.empty((B, S, D), np.float32)
    for c in range(N_CORES):
        b, h = c // 2, c % 2
        for k, g in enumerate(OWN_TILES[h]):
            out[b, g * P:(g + 1) * P, :] = res.results[c]["out"][k * P:(k + 1) * P, :]
    return out
